# revision 1
# baseline (speedup 1.0000x reference)
"""Trainium2 Bass kernel for nn_Attn (additive attention energies + softmax).

Reference computation (per batch b):
    c[g]      = sum_h Wh[g,h] * hidden[b,h] + bias[g]          (Wh = W[:, :H])
    pre[t,g]  = tanh(c[g] + sum_h enc[b,t,h] * We[g,h])        (We = W[:, H:])
    en[t]     = sum_g pre[t,g] * v[g]
    out[b,t]  = softmax_t(en)

Shapes: H=1024, B=32, T=1024.  Sharding: data-parallel over batch across 8
cores (4 batches per core); W/bias/v replicated.

Strategy vs the v1 baseline (352 us): v1 spent ~105 us of TensorE time on
PE-mode transposes (enc, W), which additionally do not count as PE-busy for
the HAM clock gate, so MM1 ran largely at the cold 1.2 GHz clock.  This
version keeps the PE on matmuls only and moves all data reshaping to the
DMA engines:
  - fp32->fp16 casts ride inside SWDGE DMAs (nc.gpsimd.dma_start) into fp16
    DRAM scratch; transposes use the DMA xbar (dma_start_transpose)
    DRAM->SBUF.  All transposes are issued on the single SP HWDGE ring:
    concurrent xbar transposes from the SP and ACT rings corrupt each
    other's data (HW-verified failure mode of this kernel's first rev).
  - MM1: psum[g=128, t=512] += WeT[h,g]^T @ encT[h,t] over 8 h-chunks.
  - ACT fuses (+c[g,b] bias, tanh) PSUM->SBUF in one pass.
  - MM2 (mm2='dve'): v[g]*tanh on DVE (per-partition tensor_scalar), fp16
    tree-add over the 8 g-chunks, then one ones-vector matmul reduces the
    128 partitions (~0.2us of PE per round instead of 1.7us).
  - Softmax over t on [4, 1024] (max-sub, Exp with fused accumulated sum,
    reciprocal, scale); energies gathered via a DRAM bounce (a direct 3D
    scatter DMA returns garbage on HW, and SBUF->SBUF DMAs concurrent with
    xbar transposes are a documented deadlock hazard).
  - In timing mode (repeat_n) the body is emitted `unroll` times inside
    For_i with per-parity weight/scratch buffers so iterations pipeline.
"""

import numpy as np

try:
    import concourse  # noqa: F401
except ImportError:  # pragma: no cover
    import sys

    sys.path.insert(0, "/opt/trn_rl_repo")

import concourse.bass as bass  # noqa: E402
import concourse.mybir as mybir  # noqa: E402
import concourse.tile as tile  # noqa: E402
from concourse import bacc  # noqa: E402
from concourse.bass_utils import run_bass_kernel_spmd  # noqa: E402

H = 1024
B = 32
T = 1024
N_CORES = 8
B_LOC = B // N_CORES  # 4 batches per core

F32 = mybir.dt.float32
F16 = mybir.dt.float16
BF16 = mybir.dt.bfloat16
AFT = mybir.ActivationFunctionType

HC = H // 128  # 8 h-chunks
GC = H // 128  # 8 g-chunks
TCH = 512  # t-chunk (PSUM one-bank limit at fp32)
N_ROUNDS = B_LOC * (T // TCH)  # 8 rounds of (batch, t-chunk)


def build_bass(repeat_n=None, mm1_dt=F16, mm2="dve", unroll=2, ablate=None):
    """Build the per-core Bass program.

    repeat_n: if set, wrap the main phase in a hardware For_i loop that
    re-executes it repeat_n times (timing mode only; results stale after
    iteration 1).  Must be divisible by `unroll`.
    mm2: 'pe' = v-reduction via 8 accumulated [1,512] matmuls;
         'dve' = v-scale + tree-add on DVE, one ones-matmul on the PE.
    """
    assert mm2 in ("pe", "dve")
    nc = bacc.Bacc("TRN2", target_bir_lowering=False, debug=False)

    enc = nc.dram_tensor("enc", [B_LOC, T, H], F32, kind="ExternalInput").ap()
    hid = nc.dram_tensor("hid", [B_LOC, H], F32, kind="ExternalInput").ap()
    w = nc.dram_tensor("w", [H, 2 * H], F32, kind="ExternalInput").ap()
    bias = nc.dram_tensor("bias", [H], F32, kind="ExternalInput").ap()
    v = nc.dram_tensor("v", [H], F32, kind="ExternalInput").ap()
    out = nc.dram_tensor("out", [B_LOC, T], F32, kind="ExternalOutput").ap()

    with tile.TileContext(nc) as tc:
        ctx_pools = []

        def pool(name, bufs, space="SBUF"):
            p = tc.tile_pool(name=name, bufs=bufs, space=space)
            ctx_pools.append(p)
            return p.__enter__()

        consts = pool("consts", 1)
        wt = pool("wt", 2)
        cpool = pool("cpool", 2)
        encTp = pool("encT", 16)
        tanhp = pool("tanh", 10)
        vredp = pool("vred", 10)
        esb = pool("esb", 2)
        wdram = pool("wdram", 2, space="DRAM")
        edram = pool("edram", 3, space="DRAM")
        # PSUM: 8 banks; 2 + 4 + 2 = 8.
        ps_c = pool("ps_c", 2, space="PSUM")
        ps_mm = pool("ps_mm", 4, space="PSUM")
        ps_en = pool("ps_en", 2, space="PSUM")

        # ---- constants (outside the timing loop, matching v1 accounting) ----
        bias_sb = consts.tile([128, GC], F32, tag="bias_sb", name="bias_sb")
        nc.sync.dma_start(bias_sb[:], bias.rearrange("(o p) -> p o", p=128))
        vf = consts.tile([128, GC], F32, tag="vf", name="vf")
        nc.sync.dma_start(vf[:], v.rearrange("(o p) -> p o", p=128))
        v16 = consts.tile([128, GC], mm1_dt, tag="v16", name="v16")
        nc.vector.tensor_copy(v16[:], vf[:])
        ones16 = consts.tile([128, 1], mm1_dt, tag="ones16", name="ones16")
        nc.vector.memset(ones16[:], 1.0)

        # hidden -> fp16, 16-partition-padded for the xbar transpose (the
        # zero padding lands in hts columns the matmul never reads anyway).
        hf = consts.tile([B_LOC, H], F32, tag="hf", name="hf")
        nc.sync.dma_start(hf[:], hid)
        h16 = consts.tile([16, H], mm1_dt, tag="h16", name="h16")
        nc.vector.memset(h16[:], 0.0)
        nc.vector.tensor_copy(h16[:B_LOC, :], hf[:])

        def emit_main():
            # Per-body tiles (parity-alternating via pool bufs=2 in timing
            # mode, so consecutive For_i bodies pipeline).
            wht = [wt.tile([128, H], mm1_dt, tag=f"wht{hc}", name=f"wht{hc}") for hc in range(HC)]
            wet = [wt.tile([128, H], mm1_dt, tag=f"wet{hc}", name=f"wet{hc}") for hc in range(HC)]
            c_sb = [cpool.tile([128, B_LOC], F32, tag=f"c{gi}", name=f"c{gi}") for gi in range(GC)]
            hts = [cpool.tile([128, 16], mm1_dt, tag=f"hts{hc}", name=f"hts{hc}") for hc in range(HC)]
            wscr = wdram.tile([H, 2 * H], mm1_dt, tag="wscr", name="wscr")
            e_stack = esb.tile([1, N_ROUNDS * TCH], F32, tag="e_stack", name="e_stack")
            energies = esb.tile([B_LOC, T], F32, tag="energies", name="energies")

            def emit_prep_cast(r):
                """SWDGE cast-DMA of one (batch, t-chunk) of enc to fp16."""
                b, tcx = divmod(r, T // TCH)
                t0 = tcx * TCH
                scr = edram.tile([TCH, H], mm1_dt, tag="escr", name="escr")
                nc.gpsimd.dma_start(scr[:], enc[b, t0 : t0 + TCH, :])
                return scr

            def emit_prep_tr(scr):
                """xbar-transpose staged enc into encT tiles, SP ring only."""
                encT = [
                    encTp.tile([128, TCH], mm1_dt, tag="encT", name="encT")
                    for _ in range(HC)
                ]
                for hc in range(HC):
                    nc.sync.dma_start_transpose(
                        encT[hc][:], scr[:, 128 * hc : 128 * (hc + 1)]
                    )
                return encT

            def emit_mm2(r, tanh_tiles):
                """v-reduction over g for round r's tanh tiles.  Deferred past
                the next round's first MM1 group so the PE never waits."""
                pen = ps_en.tile([1, TCH], F32, tag="ps_en", name="ps_en")
                if mm2 == "pe":
                    for gi in range(GC):
                        nc.tensor.matmul(
                            pen[:],
                            v16[:, gi : gi + 1],
                            tanh_tiles[gi][:],
                            start=(gi == 0),
                            stop=(gi == GC - 1),
                        )
                else:
                    lvl = []
                    for gi in range(GC):
                        m = vredp.tile([128, TCH], F16, tag="vmul", name="vmul")
                        nc.vector.tensor_scalar_mul(
                            m[:], tanh_tiles[gi][:], vf[:, gi : gi + 1]
                        )
                        lvl.append(m)
                    while len(lvl) > 1:
                        nxt = []
                        for i in range(0, len(lvl), 2):
                            o = vredp.tile([128, TCH], F16, tag="vadd", name="vadd")
                            nc.vector.tensor_add(o[:], lvl[i][:], lvl[i + 1][:])
                            nxt.append(o)
                        lvl = nxt
                    nc.tensor.matmul(pen[:], ones16[:], lvl[0][:], start=True, stop=True)
                nc.scalar.copy(e_stack[:, TCH * r : TCH * (r + 1)], pen[:])

            # SWDGE casts first so they own the DMA-completion sem lanes and
            # the Pool ring before anything else queues.
            for gi in range(GC):
                nc.gpsimd.dma_start(
                    wscr[128 * gi : 128 * (gi + 1), :],
                    w[128 * gi : 128 * (gi + 1), :],
                )
            scr0 = emit_prep_cast(0)
            # W transposes: Wh slices first (they gate the c computation,
            # which sits ahead of MM1 in the PE's in-order queue), then the
            # tiny hts transposes, then We.  Single ring only: concurrent
            # dual-ring xbar transposes corrupt each other.
            for i, c in enumerate(list(range(HC)) + list(range(HC, 2 * HC))):
                dst = wht[c] if c < HC else wet[c - HC]
                nc.sync.dma_start_transpose(dst[:], wscr[:, 128 * c : 128 * (c + 1)])
                if i == HC - 1:
                    for hc in range(HC):
                        nc.sync.dma_start_transpose(
                            hts[hc][:], h16[:, 128 * hc : 128 * (hc + 1)]
                        )
            encT_cur = emit_prep_tr(scr0)
            if ablate == "dma_only":
                for r in range(1, N_ROUNDS):
                    s_ = emit_prep_cast(r)
                    emit_prep_tr(s_)
                nc.sync.dma_start(out, energies[:])
                return
            # c[g, b] = sum_h Wh[g,h] h[b,h] + bias[g], per g-chunk
            for gi in range(GC):
                pc = ps_c.tile([128, B_LOC], F32, tag="ps_c", name="ps_c")
                for hc in range(HC):
                    nc.tensor.matmul(
                        pc[:],
                        wht[hc][:, 128 * gi : 128 * (gi + 1)],
                        hts[hc][:, :B_LOC],
                        start=(hc == 0),
                        stop=(hc == HC - 1),
                    )
                nc.vector.tensor_scalar_add(c_sb[gi][:], pc[:], bias_sb[:, gi : gi + 1])

            pending_mm2 = None
            for r in range(N_ROUNDS):
                b, tcx = divmod(r, T // TCH)
                tanh_tiles = []
                for gi in range(GC):
                    pm = ps_mm.tile([128, TCH], F32, tag="ps_mm", name="ps_mm")
                    for hc in range(HC):
                        nc.tensor.matmul(
                            pm[:],
                            wet[hc][:, 128 * gi : 128 * (gi + 1)],
                            encT_cur[hc][:],
                            start=(hc == 0),
                            stop=(hc == HC - 1),
                        )
                    th = tanhp.tile([128, TCH], F16, tag="tanh", name="tanh")
                    nc.scalar.activation(
                        th[:], pm[:], AFT.Tanh, bias=c_sb[gi][:, b : b + 1], scale=1.0
                    )
                    tanh_tiles.append(th)
                    if gi == 0 and pending_mm2 is not None:
                        emit_mm2(*pending_mm2)
                        pending_mm2 = None
                    if ablate != "pe_only":
                        if gi == 1 and r + 1 < N_ROUNDS:
                            scr_next = emit_prep_cast(r + 1)
                        if gi == 3 and r + 1 < N_ROUNDS:
                            encT_next = emit_prep_tr(scr_next)
                pending_mm2 = (r, tanh_tiles)
                if ablate != "pe_only" and r + 1 < N_ROUNDS:
                    encT_cur = encT_next
            emit_mm2(*pending_mm2)

            # gather energies [1, (b tcx t)] -> [b, (tcx t)] via a DRAM
            # bounce: a direct 3D scatter DMA returns garbage on HW, and an
            # SBUF->SBUF DMA concurrent with xbar transposes is a documented
            # HW deadlock hazard.
            edr = wdram.tile([B_LOC, T], F32, tag="edr", name="edr")
            for b in range(B_LOC):
                nc.sync.dma_start(
                    edr[b : b + 1, :], e_stack[:, T * b : T * (b + 1)]
                )
            nc.sync.dma_start(energies[:], edr[:])

            # ---- softmax over t for all 4 batches at once ----
            mx = esb.tile([B_LOC, 1], F32, tag="mx", name="mx")
            nc.vector.reduce_max(mx[:], energies[:], axis=mybir.AxisListType.X)
            nmx = esb.tile([B_LOC, 1], F32, tag="nmx", name="nmx")
            nc.vector.tensor_scalar_mul(nmx[:], mx[:], -1.0)
            ex = esb.tile([B_LOC, T], F32, tag="ex", name="ex")
            sm = esb.tile([B_LOC, 1], F32, tag="sm", name="sm")
            nc.scalar.activation(
                ex[:], energies[:], AFT.Exp, bias=nmx[:], scale=1.0, accum_out=sm[:]
            )
            rs = esb.tile([B_LOC, 1], F32, tag="rs", name="rs")
            nc.vector.reciprocal(rs[:], sm[:])
            osb = esb.tile([B_LOC, T], F32, tag="osb", name="osb")
            nc.vector.tensor_scalar_mul(osb[:], ex[:], rs[:])
            nc.sync.dma_start(out, osb[:])

        if repeat_n:
            assert repeat_n % unroll == 0, (repeat_n, unroll)
            with tc.For_i(0, repeat_n // unroll, 1):
                for _ in range(unroll):
                    emit_main()
        else:
            emit_main()

        for p in reversed(ctx_pools):
            p.__exit__(None, None, None)

    nc.compile()
    return nc


_NC = None


def _get_nc():
    global _NC
    if _NC is None:
        _NC = build_bass()
    return _NC


def kernel(hidden, encoder_outputs, W, b, v):
    nc = _get_nc()
    hidden = np.asarray(hidden, dtype=np.float32)
    encoder_outputs = np.asarray(encoder_outputs, dtype=np.float32)
    W = np.asarray(W, dtype=np.float32)
    b = np.asarray(b, dtype=np.float32)
    v = np.asarray(v, dtype=np.float32)
    hid = hidden[0]  # [B, H]
    in_maps = []
    for i in range(N_CORES):
        s = slice(B_LOC * i, B_LOC * (i + 1))
        in_maps.append(
            {
                "enc": np.ascontiguousarray(encoder_outputs[s]),
                "hid": np.ascontiguousarray(hid[s]),
                "w": W,
                "bias": b,
                "v": v,
            }
        )
    res = run_bass_kernel_spmd(nc, in_maps, core_ids=list(range(N_CORES)))
    full = np.concatenate([res.results[i]["out"] for i in range(N_CORES)], axis=0)
    return full[:, None, :].astype(np.float32)

